# revision 26
# baseline (speedup 1.0000x reference)
"""GCN 2-layer kernel for TRN2 x8 cores — host prep + Bass/Tile builder.

Math: out1 = relu(dinv ⊙ (Aᵀ (dinv ⊙ (x@W1))) + b1)
      out2 = relu(dinv ⊙ (Aᵀ (dinv ⊙ out1)) @ W2 + b2)
with A = adjacency incl. self-loops, dinv = rsqrt(in-degree incl self).

Device plan (SPMD, 8 cores, one program), v5:
- nodes dst-sharded by core (NPC per core, BLK = padded block); table
  layout row_of(g) = (g//NPC)*BLK + g%NPC, split at RSPLIT into two
  int16-addressable ranges.
- dense: each core computes only ITS block of x~@W1, one AllGather
  builds table1 [TROWS, F].
- edges (NO self loops) bucketed by (src range s, dst 64-col
  sub-window); chunks of 128 edges. Chunk counts shared across cores
  (max over cores); pad slots gather row 0 with a zero one-hot row.
- messages fetched with dma_gather (bf16 256B rows) alternating 2 SWDGE
  queues (Q7 descriptor generation is the bottleneck; 2 queues overlap).
- per-chunk one-hot S [128, 64] built ON DEVICE: DVE is_equal(iota64,
  dstcol[:, chunk]) — no S streaming from DRAM (keeps DMA free for
  gather drain). Pad slots have dstcol = -1 -> zero row.
- segment-sum via PE: lhsT = msgs [128 slots, 128 f] stationary,
  rhs = S [128, 64] -> accumulate z^T [128 f, 512 cols] in PSUM windows
  (3 PSUM buffers deep to keep window pipeline full).
- self-loop term: layer1 opens each window with z += W1ᵀ x~ᵀ[:, own
  cols]; layer2 with z += I × h2b[:, cols] (own table2 block in SBUF).
- layer1 tail: h2b = dinv*relu(dinv*z1+b1) -> transpose -> ag_in;
  AllGather -> table2; layer2 out2^T = relu(W2ᵀ qᵀ + b2).
"""
import numpy as np
import ml_dtypes

BF16 = ml_dtypes.bfloat16


# ---------------------------------------------------------------- structure
class Struct:
    pass


def make_structure(N, NC, WIN=512, SUB=64):
    P = Struct()
    P.N, P.NC, P.WIN, P.SUB = N, NC, WIN, SUB
    assert N % NC == 0
    P.NPC = N // NC
    P.BLK = ((P.NPC + 1 + 31) // 32) * 32
    assert P.BLK % SUB == 0 and P.BLK % 128 == 0
    P.RSPLIT_CORE = NC // 2
    P.RSPLIT = P.RSPLIT_CORE * P.BLK
    P.TROWS = NC * P.BLK
    assert P.RSPLIT <= 32768 and P.TROWS - P.RSPLIT <= 32768
    P.NSW = P.BLK // SUB
    P.windows = []
    col0 = 0
    while col0 < P.BLK:
        ncols = min(WIN, P.BLK - col0)
        w = Struct()
        w.col0, w.ncols = col0, ncols
        w.sw0, w.nsw = col0 // SUB, ncols // SUB
        P.windows.append(w)
        col0 += ncols
    return P


# ---------------------------------------------------------------- host prep
def prep(P, x, edge_index, W1, b1, W2, b2):
    """Returns in_maps — the per-core input dict list. Also fills P.plan."""
    N, NC, NPC, BLK, SUB = P.N, P.NC, P.NPC, P.BLK, P.SUB
    F = x.shape[1]
    HID = W1.shape[1]
    DOUT = W2.shape[1]
    P.F, P.HID, P.DOUT = F, HID, DOUT

    src = np.asarray(edge_index[0], np.int64)
    dst = np.asarray(edge_index[1], np.int64)
    deg = np.bincount(dst, minlength=N).astype(np.float64) + 1.0
    dinv = (1.0 / np.sqrt(deg)).astype(np.float32)

    # src row mapping (NO self loops in the edge stream)
    row_of = (src // NPC) * BLK + (src % NPC)
    in_range_b = (row_of >= P.RSPLIT).astype(np.int64)
    src_local = np.where(in_range_b == 1, row_of - P.RSPLIT, row_of)

    dst_core = dst // NPC
    dst_local = dst % NPC

    swglob = dst_local // SUB
    NSW = P.NSW
    key = in_range_b * NSW + swglob  # [E], in 0..2*NSW

    counts = np.zeros((NC, 2 * NSW), np.int64)
    for c in range(NC):
        m = dst_core == c
        counts[c] = np.bincount(key[m], minlength=2 * NSW)
    maxcnt = counts.max(axis=0)
    nchunks_key = (maxcnt + 127) // 128  # [2*NSW]

    P.plan = []
    for w in P.windows:
        per_s = []
        for s in range(2):
            bases = []
            for sw in range(w.sw0, w.sw0 + w.nsw):
                bases += [(sw - w.sw0) * SUB] * int(nchunks_key[s * NSW + sw])
            per_s.append(bases)
        P.plan.append(per_s)
    P.NCH = [[len(P.plan[wi][s]) for s in range(2)]
             for wi in range(len(P.windows))]
    P.TOTCH = sum(sum(n) for n in P.NCH)
    P.SLOTS = [sum(P.NCH[wi][s] for wi in range(len(P.windows)))
               * 128 for s in range(2)]

    key_off = np.zeros(2 * NSW, np.int64)
    key_choff = np.zeros(2 * NSW, np.int64)
    off_s = [0, 0]
    choff = 0
    for wi, w in enumerate(P.windows):
        for s in range(2):
            for sw in range(w.sw0, w.sw0 + w.nsw):
                k = s * NSW + sw
                key_off[k] = off_s[s]
                off_s[s] += int(nchunks_key[k]) * 128
                key_choff[k] = choff
                choff += int(nchunks_key[k])
    assert off_s[0] == P.SLOTS[0] and off_s[1] == P.SLOTS[1]
    assert choff == P.TOTCH

    # x~^T in block layout, bf16
    xT = np.zeros((F, P.TROWS), np.float32)
    xs = (x.astype(np.float32) * dinv[:, None]).T  # [F, N]
    for c in range(NC):
        xT[:, c * BLK: c * BLK + NPC] = xs[:, c * NPC:(c + 1) * NPC]
    xT = xT.astype(BF16)

    def wrap_idxs(flat):
        Sn = len(flat)
        assert Sn % 16 == 0
        w16 = flat.reshape(Sn // 16, 16).T  # [16, S/16]
        return np.tile(w16, (8, 1)).astype(np.int16)

    in_maps = []
    for c in range(NC):
        m = dst_core == c
        k_c = key[m]
        sl_c = src_local[m]
        dl_c = dst_local[m]
        order = np.argsort(k_c, kind="stable")
        k_c, sl_c, dl_c = k_c[order], sl_c[order], dl_c[order]
        cnt_c = np.bincount(k_c, minlength=2 * NSW)
        starts = np.zeros(2 * NSW, np.int64)
        starts[1:] = np.cumsum(cnt_c)[:-1]
        rank = np.arange(len(k_c)) - starts[k_c]
        slot = key_off[k_c] + rank
        s_of = (k_c >= NSW).astype(np.int64)

        idx_streams = []
        for s in range(2):
            st = np.zeros(P.SLOTS[s], np.int64)  # pad -> row 0 (S row is 0)
            ms = s_of == s
            st[slot[ms]] = sl_c[ms]
            idx_streams.append(st)

        Sm = np.zeros((P.TOTCH, 128, SUB), np.float32)
        ch_glob = key_choff[k_c] + rank // 128
        Sm[ch_glob, rank % 128, dl_c % SUB] = 1.0
        Sm = np.ascontiguousarray(Sm.transpose(1, 0, 2)).reshape(
            128, P.TOTCH * SUB).astype(BF16)

        dinvb = np.zeros((128, BLK), np.float32)
        dinvb[:, :NPC] = dinv[c * NPC:(c + 1) * NPC][None, :]
        ident = np.eye(128, dtype=np.float32)

        in_maps.append({
            "xTown": np.ascontiguousarray(xT[:, c * BLK:(c + 1) * BLK]),
            "w1": W1.astype(np.float32).astype(BF16),
            "w2": W2.astype(np.float32).astype(BF16),
            "b1": b1.astype(np.float32).reshape(HID, 1),
            "b2": b2.astype(np.float32).reshape(DOUT, 1),
            "dinvb": dinvb,
            "ident": ident.astype(BF16),
            "sall": Sm,
            "idxA": wrap_idxs(idx_streams[0]),
            "idxB": wrap_idxs(idx_streams[1]),
        })
    return in_maps


def postprocess(P, results):
    out = np.zeros((P.N, P.DOUT), np.float32)
    for c in range(P.NC):
        blk = results[c]["out"]  # [DOUT, BLK]
        out[c * P.NPC:(c + 1) * P.NPC] = blk[:, :P.NPC].T
    return out


# ---------------------------------------------------------------- builder
def build(P):
    import concourse.bacc as bacc
    import concourse.tile as tile
    import concourse.mybir as mybir

    dt = mybir.dt
    NC, BLK, SUB = P.NC, P.BLK, P.SUB
    F, HID, DOUT = P.F, P.HID, P.DOUT
    SA16 = P.SLOTS[0] // 16
    SB16 = P.SLOTS[1] // 16

    nc = bacc.Bacc("TRN2", target_bir_lowering=False, debug=False,
                   num_devices=NC, num_swdge_queues=2,
                   dynamic_dma_scratch_size=24576)
    xTown_d = nc.dram_tensor("xTown", [F, BLK], dt.bfloat16,
                             kind="ExternalInput")
    w1_d = nc.dram_tensor("w1", [F, HID], dt.bfloat16, kind="ExternalInput")
    w2_d = nc.dram_tensor("w2", [HID, DOUT], dt.bfloat16,
                          kind="ExternalInput")
    b1_d = nc.dram_tensor("b1", [HID, 1], dt.float32, kind="ExternalInput")
    b2_d = nc.dram_tensor("b2", [DOUT, 1], dt.float32, kind="ExternalInput")
    dinvb_d = nc.dram_tensor("dinvb", [128, BLK], dt.float32,
                             kind="ExternalInput")
    ident_d = nc.dram_tensor("ident", [128, 128], dt.bfloat16,
                             kind="ExternalInput")
    SCOLS = P.TOTCH * SUB
    MAXSW = max((P.NCH[wi][0] + P.NCH[wi][1]) * SUB
                for wi in range(len(P.windows)))
    sall_d = nc.dram_tensor("sall", [128, SCOLS], dt.bfloat16,
                            kind="ExternalInput")
    idxA_d = nc.dram_tensor("idxA", [128, SA16], dt.int16,
                            kind="ExternalInput")
    idxB_d = nc.dram_tensor("idxB", [128, SB16], dt.int16,
                            kind="ExternalInput")
    out_d = nc.dram_tensor("out", [DOUT, BLK], dt.float32,
                           kind="ExternalOutput")

    with tile.TileContext(nc) as tc:
        with (
            tc.tile_pool(name="dram", bufs=1, space="DRAM") as dram,
            tc.tile_pool(name="const", bufs=1) as cpool,
            tc.tile_pool(name="xchunk", bufs=3) as xpool,
            tc.tile_pool(name="dcopy", bufs=4) as dcpool,
            tc.tile_pool(name="msgs", bufs=4) as mpool,
            tc.tile_pool(name="smat", bufs=4) as spool,
            tc.tile_pool(name="drain", bufs=2) as drpool,
            tc.tile_pool(name="rows", bufs=3) as rpool,
            tc.tile_pool(name="psum_dense", bufs=2, space="PSUM") as pdense,
            tc.tile_pool(name="psum_z", bufs=4, space="PSUM") as pz,
            tc.tile_pool(name="psum_t", bufs=1, space="PSUM") as pt,
        ):
            ag1_in = dram.tile([BLK, F], dt.bfloat16)
            table1 = dram.tile([P.TROWS, F], dt.bfloat16,
                               addr_space="Shared")
            ag_in = dram.tile([BLK, HID], dt.bfloat16)
            table2 = dram.tile([P.TROWS, HID], dt.bfloat16,
                               addr_space="Shared")

            # ---- constants to SBUF
            w1sb = cpool.tile([F, HID], dt.bfloat16)
            nc.sync.dma_start(w1sb[:], w1_d[:])
            w2sb = cpool.tile([HID, DOUT], dt.bfloat16)
            nc.sync.dma_start(w2sb[:], w2_d[:])
            b1sb = cpool.tile([HID, 1], dt.float32)
            nc.sync.dma_start(b1sb[:], b1_d[:])
            b2sb = cpool.tile([DOUT, 1], dt.float32)
            nc.sync.dma_start(b2sb[:], b2_d[:])
            dinvb = cpool.tile([128, BLK], dt.float32)
            nc.sync.dma_start(dinvb[:], dinvb_d[:])
            ident = cpool.tile([128, 128], dt.bfloat16)
            nc.sync.dma_start(ident[:], ident_d[:])

            idxA = cpool.tile([128, SA16], dt.int16)
            nc.sync.dma_start(idxA[:], idxA_d[:])
            idxB = cpool.tile([128, SB16], dt.int16)
            nc.sync.dma_start(idxB[:], idxB_d[:])
            h2b = cpool.tile([128, BLK], dt.bfloat16)

            def ag(src_ap, dst_tile):
                nc.gpsimd.collective_compute(
                    "AllGather",
                    mybir.AluOpType.bypass,
                    ins=[src_ap.opt()],
                    outs=[dst_tile.opt()],
                    replica_groups=[list(range(NC))],
                )

            # ---- dense: own block of table1 = x~ @ W1, then AllGather
            NBT = BLK // 128
            XC = 8
            for t0 in range(0, NBT, XC):
                ntile = min(XC, NBT - t0)
                xc = xpool.tile([128, XC * 128], dt.bfloat16, tag="xc")
                nc.sync.dma_start(
                    xc[:, : ntile * 128],
                    xTown_d[:, t0 * 128:(t0 + ntile) * 128]
                )
                for j in range(ntile):
                    t = t0 + j
                    ps = pdense.tile([128, HID], dt.float32, tag="pd")
                    nc.tensor.matmul(
                        ps[:], xc[:, j * 128:(j + 1) * 128], w1sb[:],
                        start=True, stop=True,
                    )
                    h1 = dcpool.tile([128, HID], dt.bfloat16, tag="h1")
                    if j % 2 == 0:
                        nc.vector.tensor_copy(h1[:], ps[:])
                    else:
                        nc.scalar.copy(h1[:], ps[:])
                    nc.sync.dma_start(
                        ag1_in[t * 128:(t + 1) * 128, :], h1[:])
            ag(ag1_in, table1)

            # ---- edge phase (shared for both layers)
            qstate = [0]

            def edge_layer(table, layer):
                tabA = table[0:P.RSPLIT, :]
                tabB = table[P.RSPLIT:P.TROWS, :]
                offs16 = [0, 0]
                soff = 0  # chunk offset into dstcol
                for wi, w in enumerate(P.windows):
                    cols = slice(w.col0, w.col0 + w.ncols)
                    ncols = w.ncols
                    nchA, nchB = P.NCH[wi]
                    nch_tot = nchA + nchB

                    zw = pz.tile([128, P.WIN], dt.float32, tag="z")
                    if layer == 1:
                        xw = xpool.tile([128, P.WIN], dt.bfloat16, tag="xw")
                        nc.sync.dma_start(xw[:, :ncols], xTown_d[:, cols])
                        nc.tensor.matmul(
                            zw[:, :ncols], w1sb[:], xw[:, :ncols],
                            start=True, stop=False,
                        )
                    else:
                        nc.tensor.matmul(
                            zw[:, :ncols], ident[:], h2b[:, cols],
                            start=True, stop=False,
                        )

                    swt = spool.tile([128, MAXSW], dt.bfloat16, tag="sw")
                    nc.sync.dma_start(
                        swt[:, : nch_tot * SUB],
                        sall_d[:, soff * SUB:(soff + nch_tot) * SUB],
                    )

                    msgs = {}
                    for s, nch, idx, tab in (
                        (0, nchA, idxA, tabA), (1, nchB, idxB, tabB),
                    ):
                        if nch == 0:
                            continue
                        msgs[s] = mpool.tile([128, nch, F], dt.bfloat16,
                                             tag=f"m{s}", name=f"msgs{s}")
                        nc.gpsimd.dma_gather(
                            msgs[s][:], tab,
                            idx[:, offs16[s]: offs16[s] + nch * 8],
                            nch * 128, nch * 128, F,
                            single_packet=False,
                            queue_num=qstate[0],
                        )
                        qstate[0] ^= 1
                        offs16[s] += nch * 8

                    k_all = 0
                    for s in (0, 1):
                        for k, base in enumerate(P.plan[wi][s]):
                            k_all += 1
                            nc.tensor.matmul(
                                zw[:, base:base + SUB],
                                msgs[s][:, k, :],
                                swt[:, (k_all - 1) * SUB:k_all * SUB],
                                start=False, stop=(k_all == nch_tot),
                            )
                    soff += nch_tot

                    if layer == 1:
                        t1 = drpool.tile([128, P.WIN], dt.float32, tag="t1")
                        nc.vector.tensor_tensor(
                            t1[:, :ncols], zw[:, :ncols], dinvb[:, cols],
                            op=mybir.AluOpType.mult,
                        )
                        t2 = drpool.tile([128, P.WIN], dt.float32, tag="t2")
                        nc.scalar.activation(
                            t2[:, :ncols], t1[:, :ncols],
                            mybir.ActivationFunctionType.Relu, bias=b1sb[:],
                        )
                        nc.vector.tensor_tensor(
                            h2b[:, cols], t2[:, :ncols], dinvb[:, cols],
                            op=mybir.AluOpType.mult,
                        )
                        for j in range(0, ncols, 128):
                            nj = min(128, ncols - j)
                            tp = pt.tile([128, 128], dt.bfloat16, tag="tp")
                            nc.tensor.transpose(
                                tp[:nj, :],
                                h2b[:, w.col0 + j: w.col0 + j + nj],
                                ident[:]
                            )
                            hr = rpool.tile([128, 128], dt.bfloat16,
                                            tag="hr")
                            nc.vector.tensor_copy(hr[:nj, :], tp[:nj, :])
                            nc.sync.dma_start(
                                ag_in[w.col0 + j: w.col0 + j + nj, :],
                                hr[:nj, :]
                            )
                    else:
                        qT = drpool.tile([128, P.WIN], dt.bfloat16, tag="qT")
                        nc.vector.tensor_tensor(
                            qT[:, :ncols], zw[:, :ncols], dinvb[:, cols],
                            op=mybir.AluOpType.mult,
                        )
                        po = pt.tile([DOUT, P.WIN], dt.float32, tag="po")
                        nc.tensor.matmul(
                            po[:, :ncols], w2sb[:], qT[:, :ncols],
                            start=True, stop=True,
                        )
                        ot = rpool.tile([DOUT, P.WIN], dt.float32, tag="ot")
                        nc.scalar.activation(
                            ot[:, :ncols], po[:, :ncols],
                            mybir.ActivationFunctionType.Relu, bias=b2sb[:],
                        )
                        nc.sync.dma_start(out_d[:, cols], ot[:, :ncols])

            edge_layer(table1, 1)
            ag(ag_in, table2)
            edge_layer(table2, 2)

    nc.compile()
    return nc


# ----------------------------------------------------------------- kernel()
_BUILD_CACHE = {}
_LAST = {}


def _get_nc(P, key, **bkw):
    ent = _BUILD_CACHE.get(key)
    if ent is None:
        ent = build(P, **bkw)
        _BUILD_CACHE[key] = ent
    return ent


def kernel(x, edge_index, W1, b1, W2, b2):
    import numpy as np
    x = np.asarray(x)
    edge_index = np.asarray(edge_index)
    N = x.shape[0]
    NC = 8
    P = make_structure(N, NC)
    in_maps = prep(P, x, edge_index, np.asarray(W1), np.asarray(b1),
                   np.asarray(W2), np.asarray(b2))
    key = (N, x.shape[1], np.asarray(W2).shape[1], P.TOTCH,
           tuple(tuple(n) for n in P.NCH))
    nc = _get_nc(P, key)
    _LAST.update(P=P, in_maps=in_maps, nc=nc)
    from concourse.bass_utils import run_bass_kernel_spmd
    res = run_bass_kernel_spmd(nc, in_maps, core_ids=list(range(NC)))
    return postprocess(P, res.results).astype(np.float32)
